# revision 19
# baseline (speedup 1.0000x reference)
"""Trainium2 Bass kernel for nn_BottomUpHTMM (hierarchical tree Markov model
log-likelihood on a complete 4-ary tree, 21845 nodes, C=32 states, M=128 labels).

Strategy (hardcoded, self-contained):
  - 8-way subtree sharding: core c owns L5 nodes [128c,128c+128), their L6
    children and leaf grandchildren. Levels 0-4 (341 nodes) are replicated.
  - All per-node state kept in SBUF in "blk" layout: (128=(child_pos*32+state),
    n_parents) so every elementwise pass uses all 128 partitions.
  - B-column gathers via one-hot (is_equal vs iota) + col-tiled PE matmuls.
  - t_eps (dim,C,C,L in the reference - 358MB) is never materialized: its two
    log-likelihood contractions collapse to per-level matmuls against constant
    (C,C,L)-derived matrices.
  - One AllGather (beta at L5) on the way up, a transpose+selector-matmul to
    slice the core's own eps at the replicated->sharded boundary on the way
    down, one final AllReduce of partial sums.
"""

import numpy as np

# ---------------- problem constants (hardcoded) ----------------
C, L, M, DEPTH = 32, 4, 128, 7
DIM = 21845
NCORES = 8
STARTS = [0, 1, 5, 21, 85, 341, 1365, 5461, 21845]

# per-core shard sizes
L5_OWN, L6_OWN, LF_OWN = 128, 512, 2048
REPL = 341  # nodes at levels 0..4
N_LAB = REPL + L5_OWN + L6_OWN + LF_OWN  # 3029 labels gathered per core

# blk levels present per core: (name, F=#columns, labf offset of level data)
# level d blk columns = (#nodes at level d on this core) / 4
BLK = {
    1: (1, 1),
    2: (4, 5),
    3: (16, 21),
    4: (64, 85),
    5: (32, REPL),            # own L5  (128 nodes)
    6: (128, REPL + 128),     # own L6  (512 nodes)
    7: (512, REPL + 640),     # own leaves (2048 nodes)
}

_CACHE = {}


# ---------------- host-side parameter prep ----------------
def _softmax(x, axis):
    x = x - x.max(axis=axis, keepdims=True)
    e = np.exp(x)
    return e / e.sum(axis=axis, keepdims=True)


def _host_prep(labels, A, B, Pi, SP):
    labels = np.asarray(labels)
    A = np.asarray(A, np.float32)
    B = np.asarray(B, np.float32)
    Pi = np.asarray(Pi, np.float32)
    SP = np.asarray(SP, np.float32)

    smA = _softmax(A, 0)
    smB = _softmax(B, 1)
    smPi = _softmax(Pi, 0)
    smSP = _softmax(SP, 0)
    A_SP = smA * smSP[None, None, :]
    logA = np.log(smA)
    logB = np.log(smB)
    logPi = np.log(smPi)
    logSP = np.log(smSP)

    # W[(l*32+j), i] = A_SP[i,j,l]
    W = np.ascontiguousarray(
        np.transpose(A_SP, (2, 1, 0)).reshape(L * C, C), np.float32)
    # W2[j, (l*32+i)] = A_SP[j,i,l]; zero-padded into per-l K=128 blocks
    # (W2p[l] has W2 in rows 32l:32l+32, zeros elsewhere) so the downward
    # matmuls can consume the full blk-layout r tile with K=128.
    W2 = np.transpose(A_SP, (0, 2, 1)).reshape(C, L * C)
    Vm = A_SP * (logA + logSP[None, None, :])
    V2 = np.transpose(Vm, (0, 2, 1)).reshape(C, L * C)
    W2p = np.zeros((L, 128, 128), np.float32)
    V2p = np.zeros((L, 128, 128), np.float32)
    for l in range(L):
        W2p[l, 32 * l:32 * l + 32, :] = W2
        V2p[l, 32 * l:32 * l + 32, :] = V2
    # BT[m, 0:32]=smB[:,m], BT[m,32:64]=logB[:,m]
    BT = np.concatenate([smB.T, logB.T], axis=1).astype(np.float32)
    # blksel[(l,i), (l',i')] = (l==l')
    blksel = np.kron(np.eye(L, dtype=np.float32), np.ones((C, C), np.float32))
    Pi_blk = smPi.T.reshape(L * C, 1).astype(np.float32)
    logPi_blk = logPi.T.reshape(L * C, 1).astype(np.float32)
    iota = np.arange(128, dtype=np.float32).reshape(128, 1)
    ident = np.eye(128, dtype=np.float32)
    ones_col = np.ones((128, 1), np.float32)
    ones_pad = np.zeros((128, 32), np.float32)
    ones_pad[0:32, :] = 1.0

    consts = dict(W=W, W2p=W2p, V2p=V2p, BT=BT, blksel=blksel,
                  Pi_blk=Pi_blk, logPi_blk=logPi_blk, iota=iota, ident=ident,
                  ones_col=ones_col, ones_pad=ones_pad)

    # per-core packed labels (as f32; values < 128 are exact) and selectors
    per_core = []
    for c in range(NCORES):
        lab = np.concatenate([
            labels[:REPL],
            labels[STARTS[5] + 128 * c: STARTS[5] + 128 * (c + 1)],
            labels[STARTS[6] + 512 * c: STARTS[6] + 512 * (c + 1)],
            labels[STARTS[7] + 2048 * c: STARTS[7] + 2048 * (c + 1)],
        ]).astype(np.float32)
        labf = np.zeros((N_LAB,), np.float32)
        labf[:lab.shape[0]] = lab
        sel = np.zeros((2, 128, 32), np.float32)
        for k in range(32):
            m = 32 * c + k
            sel[m // 128, m % 128, k] = 1.0
        per_core.append((labf, sel))
    return consts, per_core


# ---------------- device program ----------------
def _build(reps=1, f32r=False, fake_ag=False, no_gather=False, no_recip=False):
    key = ("nc", reps, f32r, fake_ag, no_gather, no_recip)
    if key in _CACHE:
        return _CACHE[key]

    import concourse.bacc as bacc
    import concourse.bass as bass
    import concourse.tile as tile
    import concourse.mybir as mybir

    f32 = mybir.dt.float32
    fr = mybir.dt.float32r if f32r else mybir.dt.float32
    Alu = mybir.AluOpType

    nc = bacc.Bacc("TRN2", target_bir_lowering=False, num_devices=NCORES)

    # --- I/O ---  (float32r params carry plain fp32 bytes; fr marks them as
    # pre-rounded for the single-pass fp32 matmul mode)
    labf_d = nc.declare_dram_parameter("labf", [N_LAB], f32, isOutput=False)
    sel_d = nc.declare_dram_parameter("sel5", [2, 128, 32], fr, isOutput=False)
    cshape = dict(W=[128, 32], W2p=[4, 128, 128], V2p=[4, 128, 128],
                  BT=[128, 64], blksel=[128, 128], Pi_blk=[128, 1],
                  logPi_blk=[128, 1], iota=[128, 1], ident=[128, 128],
                  ones_col=[128, 1], ones_pad=[128, 32])
    FR_CONSTS = {"W", "W2p", "V2p", "BT", "blksel", "ident"}
    cd = {k: nc.declare_dram_parameter(k, v, fr if k in FR_CONSTS else f32,
                                       isOutput=False)
          for k, v in cshape.items()}
    ll_d = nc.dram_tensor("ll", [1, 2], f32, kind="ExternalOutput")

    # collective bounce buffers
    ag_in = nc.dram_tensor("ag_in", [128, 32], fr)
    ag_out = nc.dram_tensor("ag_out", [NCORES, 128, 32], fr,
                            addr_space="Shared")

    with tile.TileContext(nc) as tc:
        with (
            tc.tile_pool(name="const", bufs=1) as constp,
            tc.tile_pool(name="big", bufs=1) as bigp,
            tc.tile_pool(name="lvl", bufs=1) as lvlp,
            tc.tile_pool(name="scrap", bufs=2) as scrapp,
            tc.tile_pool(name="ps", bufs=4, space="PSUM") as psp,
            tc.tile_pool(name="psg", bufs=2, space="PSUM") as psgp,
        ):
            # ---- load constants ----
            cs = {}
            for k, shp in cshape.items():
                dt_k = fr if k in FR_CONSTS else f32
                if len(shp) == 3:
                    t = constp.tile([shp[1], shp[0], shp[2]], dt_k,
                                    tag=f"c_{k}")
                    nc.sync.dma_start(out=t[:],
                                      in_=cd[k][:].rearrange("l p q -> p l q"))
                else:
                    t = constp.tile(shp, dt_k, tag=f"c_{k}")
                    nc.sync.dma_start(out=t[:], in_=cd[k][:])
                cs[k] = t
            sel_t = constp.tile([128, 2, 32], fr, tag="c_sel")
            nc.sync.dma_start(
                out=sel_t[:], in_=sel_d[:].rearrange("c p k -> p c k"))

            prev = None
            for _rep in range(reps):
                prev = _emit_once(
                    nc, tc, bass, mybir, cs, sel_t, labf_d, ll_d,
                    ag_in, ag_out, f32r, fake_ag, no_gather, no_recip,
                    bigp, lvlp, scrapp, psp, psgp, prev_out=prev)

    nc.compile()
    _CACHE[key] = nc
    return nc


def _emit_once(nc, tc, bass, mybir, cs, sel_t, labf_d, ll_d,
               ag_in, ag_out, f32r, fake_ag, no_gather, no_recip,
               bigp, lvlp, scrapp, psp, psgp, prev_out=None):
    f32 = mybir.dt.float32
    fr = mybir.dt.float32r if f32r else mybir.dt.float32
    Alu = mybir.AluOpType
    if True:
        if True:
            # ---- broadcast labels + one-hot ----
            iota_ap = cs["iota"][:, 0:1]
            if prev_out is not None:
                # serialize reps: make this rep's one-hot depend on the
                # previous rep's output so no rep is dead code
                car = lvlp.tile([128, 1], f32, tag="car")
                nc.gpsimd.partition_broadcast(car[:], prev_out[0:1, 0:1])
                iota_dep = lvlp.tile([128, 1], f32, tag="iota_dep")
                nc.vector.scalar_tensor_tensor(
                    out=iota_dep[:], in0=car[:], scalar=0.0,
                    in1=cs["iota"][:, 0:1], op0=Alu.mult, op1=Alu.add)
                iota_ap = iota_dep[:, 0:1]
            if not no_gather:
                labbc = bigp.tile([128, N_LAB], f32, tag="labbc")
                lab_ap = labf_d[:]
                bcast_src = bass.AP(tensor=lab_ap.tensor, offset=lab_ap.offset,
                                    ap=[[0, 128]] + list(lab_ap.ap))
                nc.gpsimd.dma_start(out=labbc[:], in_=bcast_src)
                onehot = bigp.tile([128, N_LAB], fr, tag="onehot")
                nc.vector.tensor_scalar(
                    out=onehot[:], in0=labbc[:], scalar1=iota_ap,
                    scalar2=None, op0=Alu.is_equal)

            # ---- gathers: Bg/logBg per level in blk layout ----
            Bg, logBg = {}, {}
            if no_gather:
                for d, (F, off) in BLK.items():
                    bg = lvlp.tile([128, F], f32, tag=f"bg{d}")
                    lbg = lvlp.tile([128, F], f32, tag=f"lbg{d}")
                    nc.vector.memset(bg[:], 0.5)
                    nc.vector.memset(lbg[:], -0.5)
                    Bg[d], logBg[d] = bg, lbg
                bg_root = lvlp.tile([32, 1], f32, tag="bg_root")
                lbg_root = lvlp.tile([32, 1], f32, tag="lbg_root")
                nc.vector.memset(bg_root[:], 0.5)
                nc.vector.memset(lbg_root[:], -0.5)
            for d, (F, off) in ({} if no_gather else BLK).items():
                pg = psgp.tile([128, 512], f32, tag="g")
                pl = psgp.tile([128, 512], f32, tag="lg")
                for l in range(L):
                    rhs = onehot[:, off + l: off + 4 * F: 4]
                    nc.tensor.matmul(
                        out=pg[32 * l:32 * l + 32, 0:F],
                        lhsT=cs["BT"][:, 0:32], rhs=rhs,
                        start=True, stop=True, tile_position=(0, 32 * l))
                    nc.tensor.matmul(
                        out=pl[32 * l:32 * l + 32, 0:F],
                        lhsT=cs["BT"][:, 32:64], rhs=rhs,
                        start=True, stop=True, tile_position=(0, 32 * l))
                bg = lvlp.tile([128, F], f32, tag=f"bg{d}")
                lbg = lvlp.tile([128, F], f32, tag=f"lbg{d}")
                nc.scalar.copy(out=bg[:], in_=pg[:, 0:F])
                nc.scalar.copy(out=lbg[:], in_=pl[:, 0:F])
                Bg[d], logBg[d] = bg, lbg
            # root (single node): two (32,1) matmuls
            if no_gather:
                pg_r = None
            else:
                pg_r = psgp.tile([128, 512], f32, tag="g")
            if not no_gather:
                pl_r = psgp.tile([128, 512], f32, tag="lg")
                nc.tensor.matmul(out=pg_r[0:32, 0:1], lhsT=cs["BT"][:, 0:32],
                                 rhs=onehot[:, 0:1], start=True, stop=True)
                nc.tensor.matmul(out=pl_r[0:32, 0:1], lhsT=cs["BT"][:, 32:64],
                                 rhs=onehot[:, 0:1], start=True, stop=True)
                bg_root = lvlp.tile([32, 1], f32, tag="bg_root")
                lbg_root = lvlp.tile([32, 1], f32, tag="lbg_root")
                nc.scalar.copy(out=bg_root[:], in_=pg_r[0:32, 0:1])
                nc.scalar.copy(out=lbg_root[:], in_=pl_r[0:32, 0:1])

            beta, tbr = {}, {}

            def recip_(out, in_):
                if no_recip:
                    nc.vector.tensor_scalar(out=out, in0=in_, scalar1=1.0,
                                            scalar2=None, op0=Alu.mult)
                else:
                    nc.vector.reciprocal(out=out, in_=in_)

            def normalize(d, bl, F):
                """beta = bl / per-node(32-block) sum, via blksel matmul."""
                pn = psp.tile([128, 512], f32, tag="mm")
                nc.tensor.matmul(out=pn[:, 0:F], lhsT=cs["blksel"][:],
                                 rhs=bl[:], start=True, stop=True)
                rcp = scrapp.tile([128, F], f32, tag="rcp")
                recip_(rcp[:], pn[:, 0:F])
                bt = lvlp.tile([128, F], fr, tag=f"beta{d}")
                nc.vector.tensor_tensor(out=bt[:], in0=bl[:], in1=rcp[:],
                                        op=Alu.mult)
                beta[d] = bt

            # ---- upward: leaves ----
            F = 512
            bl7 = scrapp.tile([128, F], fr, tag="bl")
            nc.vector.tensor_scalar(out=bl7[:], in0=Bg[7][:],
                                    scalar1=cs["Pi_blk"][:, 0:1], scalar2=None,
                                    op0=Alu.mult)
            normalize(7, bl7, F)

            # ---- upward levels 6..1 ----
            btbr = {}

            def up_level(d, child_beta, F):
                ptb = psp.tile([128, 512], f32, tag="mm")
                for l in range(L):
                    nc.tensor.matmul(
                        out=ptb[32 * l:32 * l + 32, 0:F], lhsT=cs["W"][:],
                        rhs=child_beta[:, l::L], start=True, stop=True,
                        tile_position=(0, 32 * l))
                tr = lvlp.tile([128, F], f32, tag=f"tbr{d}")
                recip_(tr[:], ptb[:, 0:F])
                tbr[d] = tr
                bl = scrapp.tile([128, F], fr, tag="bl")
                nc.vector.tensor_tensor(out=bl[:], in0=ptb[:, 0:F], in1=Bg[d][:],
                                        op=Alu.mult)
                normalize(d, bl, F)
                if d <= 6:
                    # beta*tbr, used by the downward pass r-chain; off the
                    # upward critical path
                    bb = lvlp.tile([128, F], fr, tag=f"btbr{d}")
                    nc.vector.tensor_tensor(out=bb[:], in0=beta[d][:],
                                            in1=tr[:], op=Alu.mult)
                    btbr[d] = bb

            up_level(6, beta[7][:], 128)
            up_level(5, beta[6][:], 32)

            # ---- AllGather beta5 (own 32 cols -> full 256 cols) ----
            nc.sync.dma_start(out=ag_in[:], in_=beta[5][:])
            if fake_ag:
                nc.sync.dma_start(out=ag_out[:], in_=bass.AP(
                    tensor=ag_in[:].tensor, offset=0,
                    ap=[[0, NCORES]] + list(ag_in[:].ap)))
            else:
                nc.gpsimd.collective_compute(
                    "AllGather", mybir.AluOpType.bypass,
                    replica_groups=[list(range(NCORES))],
                    ins=[ag_in[:]], outs=[ag_out[:]])
            b5full = lvlp.tile([128, NCORES, 32], fr, tag="b5full")
            nc.sync.dma_start(out=b5full[:],
                              in_=ag_out[:].rearrange("c p k -> p c k"))
            b5full_f = b5full[:].rearrange("p c k -> p (c k)")

            up_level(4, b5full_f, 64)
            up_level(3, beta[4][:], 16)
            up_level(2, beta[3][:], 4)
            up_level(1, beta[2][:], 1)

            # ---- root ----
            ptb0 = psp.tile([128, 512], f32, tag="mm")
            nc.tensor.matmul(out=ptb0[0:32, 0:1], lhsT=cs["W"][:], rhs=beta[1][:],
                             start=True, stop=True)
            tbr0 = lvlp.tile([32, 1], f32, tag="tbr0")
            recip_(tbr0[:], ptb0[0:32, 0:1])
            bl0 = lvlp.tile([128, 1], f32, tag="bl0")
            nc.vector.memset(bl0[:], 0.0)
            nc.vector.tensor_tensor(out=bl0[0:32, :], in0=ptb0[0:32, 0:1],
                                    in1=bg_root[:], op=Alu.mult)
            ps0 = psp.tile([128, 512], f32, tag="mm")
            nc.tensor.matmul(out=ps0[0:32, 0:1], lhsT=cs["ones_pad"][:],
                             rhs=bl0[:], start=True, stop=True)
            rcp0 = lvlp.tile([32, 1], f32, tag="rcp0")
            recip_(rcp0[:], ps0[0:32, 0:1])
            beta0 = lvlp.tile([32, 1], f32, tag="beta0")
            nc.vector.tensor_tensor(out=beta0[:], in0=bl0[0:32, :], in1=rcp0[:],
                                    op=Alu.mult)

            # ---- accumulators ----
            accL = lvlp.tile([128, 10], f32, tag="accL")
            accS = lvlp.tile([128, 6], f32, tag="accS")
            # the root-t2 accum writes only partitions 0:32 of its column
            nc.vector.memset(accL[:], 0.0)
            acc_cols = {"L": 0, "S": 0}

            def accum_stt(in0, in1, which, np_=128):
                t = accL if which == "L" else accS
                k = acc_cols[which]
                acc_cols[which] += 1
                fsz = 1
                for dd in in0.shape[1:]:
                    fsz *= dd
                sc = scrapp.tile([np_, fsz], f32, tag="sttscrap")
                scv = sc[:] if len(in0.shape) == 2 else sc[:].rearrange(
                    "x (p l) -> x p l", l=in0.shape[-1])
                nc.vector.scalar_tensor_tensor(
                    out=scv, in0=in0, scalar=1.0, in1=in1, op0=Alu.bypass,
                    op1=Alu.mult, accum_out=t[0:np_, k:k + 1])

            # ---- downward ----
            eps = {}
            r0 = lvlp.tile([128, 1], fr, tag="r0")
            nc.vector.memset(r0[:].bitcast(f32), 0.0)
            nc.vector.tensor_tensor(out=r0[0:32, :], in0=beta0[:], in1=tbr0[:],
                                    op=Alu.mult)

            def down_matmuls(r_ap, F_par, tag):
                """r (128, F_par) blk -> Q,VQ for 4*F_par children.

                Quarter l (columns [l*F_par:(l+1)*F_par]) holds children of
                nodes m=4p+l. Returned views are rearranged to (p, l) so the
                free iteration order matches blk column order m=4p+l."""
                pq = psp.tile([128, 512], f32, tag="mm")
                pv = psp.tile([128, 512], f32, tag="mm")
                for l in range(L):
                    nc.tensor.matmul(
                        out=pq[:, l * F_par:(l + 1) * F_par],
                        lhsT=cs["W2p"][:, l, :], rhs=r_ap,
                        start=(l == 0), stop=(l == 3))
                for l in range(L):
                    nc.tensor.matmul(
                        out=pv[:, l * F_par:(l + 1) * F_par],
                        lhsT=cs["V2p"][:, l, :], rhs=r_ap,
                        start=(l == 0), stop=(l == 3))
                pq_v = pq[:, 0:4 * F_par].rearrange(
                    "x (l p) -> x p l", l=4)
                pv_v = pv[:, 0:4 * F_par].rearrange(
                    "x (l p) -> x p l", l=4)
                return pq_v, pv_v

            # dp=0 (root -> blk1); r0 is (32,1): single matmuls, no tiling
            pq = psp.tile([128, 512], f32, tag="mm")
            pv = psp.tile([128, 512], f32, tag="mm")
            nc.tensor.matmul(out=pq[:, 0:1], lhsT=cs["W2p"][:, 0, :], rhs=r0[:],
                             start=True, stop=True)
            nc.tensor.matmul(out=pv[:, 0:1], lhsT=cs["V2p"][:, 0, :], rhs=r0[:],
                             start=True, stop=True)
            e1 = lvlp.tile([128, 1], f32, tag="eps1")
            nc.vector.tensor_tensor(out=e1[:], in0=beta[1][:], in1=pq[:, 0:1],
                                    op=Alu.mult)
            eps[1] = e1
            accum_stt(beta[1][:], pv[:, 0:1], "L")
            accum_stt(e1[:], logBg[1][:], "L")
            accum_stt(beta0[:], lbg_root[:], "L", np_=32)
            rq_1 = pq[:, 0:1].rearrange("x (p l) -> x p l", l=1)

            # dp=1..3 fully replicated (children blk2..blk4)
            # chain: rq[dp] (=r in blk) -> Q matmul -> rq[dp+1]; eps (for t2)
            # is computed off-chain as beta*Q.
            rq = {1: rq_1}
            for dp in range(1, 4):
                Fp = BLK[dp][0]
                rr = scrapp.tile([128, Fp], fr, tag=f"r{dp}")
                if dp == 1:
                    bb = btbr[dp][:].rearrange("x (p l) -> x p l", l=1)
                    rrv = rr[:].rearrange("x (p l) -> x p l", l=1)
                else:
                    bb = btbr[dp][:].rearrange("x (p l) -> x p l", l=4)
                    rrv = rr[:].rearrange("x (p l) -> x p l", l=4)
                nc.vector.tensor_tensor(out=rrv, in0=bb, in1=rq[dp],
                                        op=Alu.mult)
                pq_v, pv_v = down_matmuls(rr[:], Fp, dp)
                rq[dp + 1] = pq_v
                ed = lvlp.tile([128, 4 * Fp], f32, tag=f"eps{dp+1}")
                bview = beta[dp + 1][:].rearrange("x (p l) -> x p l", l=4)
                nc.vector.tensor_tensor(
                    out=ed[:].rearrange("x (p l) -> x p l", l=4),
                    in0=bview, in1=pq_v, op=Alu.mult)
                eps[dp + 1] = ed
                accum_stt(bview, pv_v, "L")
                accum_stt(ed[:], logBg[dp + 1][:], "L")

            # dp=4: children = full L5 (256 cols), still replicated
            rr = scrapp.tile([128, 64], fr, tag="r4")
            nc.vector.tensor_tensor(
                out=rr[:].rearrange("x (p l) -> x p l", l=4),
                in0=btbr[4][:].rearrange("x (p l) -> x p l", l=4),
                in1=rq[4], op=Alu.mult)
            pq_v, pv_v = down_matmuls(rr[:], 64, 4)
            rq[5] = pq_v
            e5full = lvlp.tile([128, 256], fr, tag="eps5full")
            b5view = b5full_f.rearrange("x (p l) -> x p l", l=4)
            nc.vector.tensor_tensor(
                out=e5full[:].rearrange("x (p l) -> x p l", l=4),
                in0=b5view, in1=pq_v, op=Alu.mult)
            accum_stt(b5view, pv_v, "L")

            # ---- boundary: slice own 32 columns of eps5 via transpose+sel ----
            peo = psp.tile([128, 512], f32, tag="mm")
            for h in range(2):
                pt = psp.tile([128, 512], f32, tag="mm")
                nc.tensor.transpose(pt[:, 0:128].bitcast(fr),
                                    e5full[:, 128 * h:128 * (h + 1)],
                                    cs["ident"][:])
                et = scrapp.tile([128, 128], fr, tag="etr")
                nc.scalar.copy(out=et[:], in_=pt[:, 0:128])
                nc.tensor.matmul(out=peo[:, 0:32], lhsT=et[:], rhs=sel_t[:, h, :],
                                 start=(h == 0), stop=(h == 1))
            e5own = lvlp.tile([128, 32], f32, tag="eps5own")
            nc.scalar.copy(out=e5own[:], in_=peo[:, 0:32])
            eps[5] = e5own
            accum_stt(e5own[:], logBg[5][:], "S")

            # dp=5, dp=6 (own shard)
            for dp in (5, 6):
                Fp = BLK[dp][0]
                rr = scrapp.tile([128, Fp], fr, tag=f"r{dp}")
                if dp == 5:
                    nc.vector.tensor_tensor(out=rr[:], in0=eps[5][:],
                                            in1=tbr[5][:], op=Alu.mult)
                else:
                    nc.vector.tensor_tensor(
                        out=rr[:].rearrange("x (p l) -> x p l", l=4),
                        in0=btbr[6][:].rearrange("x (p l) -> x p l", l=4),
                        in1=rq[6], op=Alu.mult)
                pq_v, pv_v = down_matmuls(rr[:], Fp, dp)
                rq[dp + 1] = pq_v
                ed = lvlp.tile([128, 4 * Fp], f32, tag=f"eps{dp+1}")
                bview = beta[dp + 1][:].rearrange("x (p l) -> x p l", l=4)
                nc.vector.tensor_tensor(
                    out=ed[:].rearrange("x (p l) -> x p l", l=4),
                    in0=bview, in1=pq_v, op=Alu.mult)
                eps[dp + 1] = ed
                accum_stt(bview, pv_v, "S")
                accum_stt(ed[:], logBg[dp + 1][:], "S")

            # t3: leaves eps * logPi (per-partition const)
            sc = scrapp.tile([128, 512], f32, tag="sttscrap")
            nc.vector.tensor_scalar(
                out=sc[:], in0=eps[7][:], scalar1=cs["logPi_blk"][:, 0:1],
                scalar2=None, op0=Alu.mult, op1=Alu.add,
                accum_out=accS[:, acc_cols["S"]:acc_cols["S"] + 1])
            acc_cols["S"] += 1

            # ---- final reduction: per-core partials, summed on host ----
            tot2 = lvlp.tile([128, 2], f32, tag="tot2")
            nc.vector.tensor_reduce(out=tot2[:, 0:1], in_=accS[:],
                                    axis=mybir.AxisListType.X, op=Alu.add)
            nc.vector.tensor_reduce(out=tot2[:, 1:2], in_=accL[:],
                                    axis=mybir.AxisListType.X, op=Alu.add)
            pfin = psp.tile([128, 512], f32, tag="mm")
            nc.tensor.matmul(out=pfin[0:1, 0:2], lhsT=cs["ones_col"][:],
                             rhs=tot2[:], start=True, stop=True)
            outsb = lvlp.tile([1, 2], f32, tag="outsb")
            nc.scalar.copy(out=outsb[:], in_=pfin[0:1, 0:2])
            nc.sync.dma_start(out=ll_d[:], in_=outsb[:])
            return outsb


# ---------------- entry point ----------------
def kernel(labels, A, B, Pi, SP):
    from concourse.bass_utils import run_bass_kernel_spmd

    consts, per_core = _host_prep(labels, A, B, Pi, SP)
    nc = _build()
    in_maps = []
    for c in range(NCORES):
        labf, sel = per_core[c]
        m = {"labf": labf, "sel5": sel}
        m.update(consts)
        in_maps.append(m)
    res = run_bass_kernel_spmd(nc, in_maps, core_ids=list(range(NCORES)))
    total = np.float32(0.0)
    for c in range(NCORES):
        total += res.results[c]["ll"][0, 0]
    total += res.results[0]["ll"][0, 1]
    return np.float32(total)
